# revision 42
# baseline (speedup 1.0000x reference)
"""Causal self-attention Bass kernel for Trainium2, 8 NeuronCores.

Problem shapes (hardcoded): x (4, 2048, 512), W_qkv (1536, 512),
W_out (512, 512), b_out (512,); NH=8 heads, DH=64.

Sharding: core c handles batch b = c // 2 and head group g = c % 2
(4 heads each). Each core computes its QKV slice, causal attention for
its 4 heads, and a partial output projection over its 256 y-dims.
Host sums the two partials per batch and adds the bias.

Per-core device pipeline (all matmul operands bf16, f32 PSUM accum):
  1. qkT projection: qkT[c_loc, t] chunks ordered [q-pair0, k-pair0,
     q-pair1, k-pair1] so each head's qT/kT share a 64-partition range.
  2. v projection directly in natural [t, d] layout, augmented with a
     ones column per head (denominator trick).
  3. Per head h, per key chunk j (128 rows): scoresT[k, q] for q >=
     128j only (causality via matmul geometry), one exp ACTIVATE per
     (h, j) with the 1/8 scale folded in, triangular mask applied to
     the 128x128 diagonal block post-exp (gpsimd multiply).
  4. att @ v with lhsT = [v | ones] (M=65): row 64 accumulates the
     softmax denominators for free.
  5. y normalization: partition-broadcast of the denominator row,
     vector reciprocal + multiply into the stacked ynormT layout.
  6. Output projection partial (K = 256 local y-dims).
"""

import sys

if "/opt/trn_rl_repo" not in sys.path:
    sys.path.insert(0, "/opt/trn_rl_repo")

import numpy as np
import ml_dtypes

B, T, D, NH, DH = 4, 2048, 512, 8, 64
HPC = 4  # heads per core
N_CORES = 8
BF16 = ml_dtypes.bfloat16

_PROG = None


def _build_program():
    import concourse.bass as bass
    import concourse.mybir as mybir
    import concourse.tile as tile
    from concourse import bacc

    f32 = mybir.dt.float32
    bf16 = mybir.dt.bfloat16
    Exp = mybir.ActivationFunctionType.Exp

    nc = bacc.Bacc("TRN2", target_bir_lowering=False, debug=False)

    xT_d = nc.dram_tensor("xT", [D, T], bf16, kind="ExternalInput").ap()
    wqkT_d = nc.dram_tensor("wqkT", [D, 512], bf16, kind="ExternalInput").ap()
    wvT_d = nc.dram_tensor("wvT", [D, 256], bf16, kind="ExternalInput").ap()
    woT_d = nc.dram_tensor("woT", [256, D], bf16, kind="ExternalInput").ap()
    triu_d = nc.dram_tensor("triu", [128, 128], bf16, kind="ExternalInput").ap()
    out_d = nc.dram_tensor("out", [T, D], bf16, kind="ExternalOutput").ap()

    NT = T // 128  # 16 key/query 128-chunks
    NQ = T // 512  # 4 query 512-chunks

    with tile.TileContext(nc) as tc:
        with (
            tc.tile_pool(name="const", bufs=1) as cpool,
            tc.tile_pool(name="att", bufs=2) as apool,
            tc.tile_pool(name="work", bufs=2) as wpool,
            tc.tile_pool(name="outsb", bufs=2) as opool,
            tc.tile_pool(name="ps", bufs=1, space="PSUM") as pspool,
        ):
            # ---- input DMAs ----
            xT_sb = cpool.tile([128, 4, T], bf16, tag="xT")
            for kc in range(4):
                nc.sync.dma_start(out=xT_sb[:, kc, :], in_=xT_d[128 * kc : 128 * (kc + 1), :])
            wqkT_sb = cpool.tile([128, 4, 512], bf16, tag="wqkT")
            for kc in range(4):
                nc.sync.dma_start(out=wqkT_sb[:, kc, :], in_=wqkT_d[128 * kc : 128 * (kc + 1), :])
            wvT_sb = cpool.tile([128, 4, 256], bf16, tag="wvT")
            for kc in range(4):
                nc.sync.dma_start(out=wvT_sb[:, kc, :], in_=wvT_d[128 * kc : 128 * (kc + 1), :])
            woT_sb = cpool.tile([128, 2, 512], bf16, tag="woT")
            for kc in range(2):
                nc.sync.dma_start(out=woT_sb[:, kc, :], in_=woT_d[128 * kc : 128 * (kc + 1), :])
            triu_sb = cpool.tile([128, 128], bf16, tag="triu")
            nc.sync.dma_start(out=triu_sb[:, :], in_=triu_d[:, :])

            # qkT chunks: [q-pair0 | k-pair0 | q-pair1 | k-pair1]; head h at
            # partitions 64*(h%2) .. +64 of chunks (2*(h//2), 2*(h//2)+1).
            qkT_sb = cpool.tile([128, 4, T], bf16, tag="qkT")
            # v_all: per t-chunk, per head: 64 v-dims + a ones column (65).
            v_all = cpool.tile([128, NT, HPC * 65], bf16, tag="v_all")
            ynormT = cpool.tile([128, 2, T], bf16, tag="ynormT")

            ones_ap = v_all[:, :, :].rearrange("p t (h e) -> p (t h) e", e=65)[:, :, 64:65]
            nc.gpsimd.memset(ones_ap, 1.0)
            # ones row at partition 64 used to broadcast the denominator row
            # via a rank-1 fp32 matmul (walrus can't codegen
            # InstPartitionBroadcast; matmul wants lhsT/rhs on the same base
            # partition as the den row, and fp32r inputs would need rounding)
            ones64 = cpool.tile([65, 64], bf16, tag="ones64")
            nc.gpsimd.memset(ones64[:, :], 1.0)

            # ---- phase A: projections. Only m=0,1 (heads 0/1 qk) and the
            # first v half run up front; the rest interleave into head 0's
            # attention as PE filler (they use ps_s slots, never ps_y).
            def emit_proj_m_half(m, half):
                ps = pspool.tile([128, 1024], f32, tag="ps_s", bufs=2)
                for n2 in range(2):
                    n = 2 * half + n2
                    for kc in range(4):
                        nc.tensor.matmul(
                            ps[:, 512 * n2 : 512 * (n2 + 1)],
                            lhsT=wqkT_sb[:, kc, 128 * m : 128 * (m + 1)],
                            rhs=xT_sb[:, kc, 512 * n : 512 * (n + 1)],
                            start=(kc == 0),
                            stop=(kc == 3),
                        )
                nc.vector.tensor_copy(
                    qkT_sb[:, m, 1024 * half : 1024 * (half + 1)], ps[:, :]
                )

            def emit_proj_v_quarter(qt):
                ps = pspool.tile([128, 1024], f32, tag="ps_s", bufs=2)
                for tl in range(4):
                    t = 4 * qt + tl
                    for kc in range(4):
                        nc.tensor.matmul(
                            ps[:, 256 * tl : 256 * (tl + 1)],
                            lhsT=xT_sb[:, kc, 128 * t : 128 * (t + 1)],
                            rhs=wvT_sb[:, kc, :],
                            start=(kc == 0),
                            stop=(kc == 3),
                        )
                dst = v_all[:, 4 * qt : 4 * (qt + 1), :].rearrange(
                    "p t (h e) -> p t h e", e=65
                )[:, :, :, 0:64]
                src = ps[:, :].rearrange("p (t h e) -> p t h e", t=4, h=HPC)
                nc.vector.tensor_copy(dst, src)

            # preload the exp table set during the input DMAs so the first
            # real ACTIVATE doesn't pay the ~2.7us ACT_TABLE_LOAD
            actwarm = wpool.tile([1, 16], f32, tag="actwarm")
            nc.scalar.activation(actwarm[:, :], triu_sb[0:1, 0:16], Exp, scale=0.125)

            emit_proj_m_half(0, 0)
            emit_proj_m_half(0, 1)
            emit_proj_m_half(1, 0)
            emit_proj_m_half(1, 1)

            # ---- phase B/C: per-head attention ----
            def emit_av(h, j, ps_y, att):
                # accumulate yT_aug[:, q] += v_augT @ attT for key chunk j
                q0 = 128 * j
                lhsT = v_all[:, j, :].rearrange("p (h e) -> p h e", e=65)[:, h, :]
                for c in range(NQ):
                    if j > 4 * c + 3:
                        continue  # fully masked: k > all q in this chunk
                    qs = max(512 * c, q0)
                    qe = 512 * (c + 1)
                    nc.tensor.matmul(
                        ps_y[:, qs:qe],
                        lhsT=lhsT,
                        rhs=att[:, qs - q0 : qe - q0],
                        start=(j == 0),
                        stop=(j == 4 * c + 3),
                    )

            def emit_norm_cols(h, c0, c1, ps_y):
                """normalize y[:, c0:c1] (512-aligned) once all avs that
                touch those columns are in (av(j) writes cols >= 128j).
                The den row is cast to bf16 so the rank-1 broadcast
                matmuls run at bf16 rate (fp32 matmuls are 4x slower on
                the PE); the broadcast lands back in the already-copied-
                out region of ps_y."""
                base = 64 * (h % 2)
                w = c1 - c0
                yT = wpool.tile([65, 1024], f32, tag="yT", name=f"yT{h}_{c0}")
                nc.vector.tensor_copy(yT[:, 0:w], ps_y[:, c0:c1])
                den = wpool.tile([1, 1024], bf16, tag="den", name=f"den{h}_{c0}")
                nc.vector.tensor_copy(den[:, 0:w], yT[64:65, 0:w])
                for c2 in range(w // 512):
                    nc.tensor.matmul(
                        ps_y[0:64, c0 + 512 * c2 : c0 + 512 * (c2 + 1)],
                        lhsT=ones64[0:1, :],
                        rhs=den[0:1, 512 * c2 : 512 * (c2 + 1)],
                        start=True,
                        stop=True,
                    )
                recb = wpool.tile([64, 1024], f32, tag="recb", name=f"recb{h}_{c0}")
                nc.vector.reciprocal_approx_fast(
                    out=recb[:, 0:w], in_=ps_y[0:64, c0:c1]
                )
                dst = ynormT[base : base + 64, h // 2, c0:c1]
                nc.vector.tensor_mul(dst, yT[0:64, 0:w], recb[:, 0:w])

            def emit_outproj(grp):
                ps = pspool.tile([128, 1024], f32, tag="ps_s", bufs=2)
                for i2 in range(2):
                    i = 2 * grp + i2
                    for kc in range(2):
                        nc.tensor.matmul(
                            ps[:, 512 * i2 : 512 * (i2 + 1)],
                            lhsT=ynormT[:, kc, 128 * i : 128 * (i + 1)],
                            rhs=woT_sb[:, kc, :],
                            start=(kc == 0),
                            stop=(kc == 1),
                        )
                osb = opool.tile([128, 1024], bf16, tag="osb")
                nc.vector.tensor_copy(osb[:, :], ps[:, :])
                for i2 in range(2):
                    i = 2 * grp + i2
                    nc.sync.dma_start(
                        out=out_d[128 * i : 128 * (i + 1), :],
                        in_=osb[:, 512 * i2 : 512 * (i2 + 1)],
                    )

            for h in range(HPC):
                base = 64 * (h % 2)
                qc = 2 * (h // 2)  # q chunk index; k chunk is qc + 1
                qT = qkT_sb[base : base + 64, qc, :]
                kT = qkT_sb[base : base + 64, qc + 1, :]
                ps_y = pspool.tile([65, T], f32, tag="ps_y")
                att_tiles = {}
                for j in range(NT):
                    q0 = 128 * j
                    att = apool.tile([128, T - q0], bf16, tag=f"att{j}")
                    # split the q range at 1024 so scores(j+1) can run in the
                    # second ps_s buffer while exp(j) drains the first
                    halves = [(q0, 1024), (1024, T)] if j < 8 else [(q0, T)]
                    for hs, he in halves:
                        ps_s = pspool.tile([128, 1024], f32, tag="ps_s", bufs=2)
                        for rel in range(0, he - hs, 512):
                            n = min(512, he - hs - rel)
                            nc.tensor.matmul(
                                ps_s[:, rel : rel + n],
                                lhsT=kT[:, 128 * j : 128 * (j + 1)],
                                rhs=qT[:, hs + rel : hs + rel + n],
                                start=True,
                                stop=True,
                            )
                        nc.scalar.activation(
                            att[:, hs - q0 : he - q0],
                            ps_s[:, 0 : he - hs],
                            Exp,
                            scale=0.125,
                        )
                    # mask the strict upper triangle of the diagonal block
                    nc.gpsimd.tensor_mul(att[:, 0:128], att[:, 0:128], triu_sb[:, :])
                    att_tiles[j] = att
                    if j >= 2:
                        emit_av(h, j - 2, ps_y, att_tiles[j - 2])
                    if h == 0:
                        # deferred projections as PE filler for the
                        # scores->exp ping-pong gaps of the first head
                        if j == 0:
                            emit_proj_v_quarter(0)
                        elif j == 1:
                            emit_proj_v_quarter(1)
                        elif j == 2:
                            emit_proj_m_half(2, 0)
                        elif j == 3:
                            emit_proj_m_half(2, 1)
                        elif j == 4:
                            emit_proj_m_half(3, 0)
                        elif j == 5:
                            emit_proj_m_half(3, 1)
                        elif j == 6:
                            emit_proj_v_quarter(2)
                        elif j == 7:
                            emit_proj_v_quarter(3)
                    if j == 11:
                        # cols 0:1024 of ps_y complete (last write av(7),
                        # emitted at j==9); remaining avs touch cols >=1280
                        emit_norm_cols(h, 0, 1024, ps_y)
                    if h == HPC - 1 and j == 13:
                        emit_outproj(0)
                        emit_outproj(1)
                if h < HPC - 1:
                    emit_av(h, NT - 2, ps_y, att_tiles[NT - 2])
                    emit_av(h, NT - 1, ps_y, att_tiles[NT - 1])
                    emit_norm_cols(h, 1024, 2048, ps_y)
                else:
                    # progressive tail: normalize 512-col quarters as their
                    # avs land so output projections start without waiting
                    # for av(15); grp g consumes ynormT cols 256g..256g+256
                    emit_outproj(2)
                    emit_norm_cols(h, 1024, 1536, ps_y)  # needs av(11)
                    emit_av(h, NT - 2, ps_y, att_tiles[NT - 2])
                    emit_outproj(3)
                    emit_outproj(4)
                    emit_av(h, NT - 1, ps_y, att_tiles[NT - 1])
                    emit_outproj(5)
                    emit_norm_cols(h, 1536, 2048, ps_y)  # needs av(15)
                    emit_outproj(6)
                    emit_outproj(7)

    nc.compile()
    return nc


def _get_program():
    global _PROG
    if _PROG is None:
        _PROG = _build_program()
    return _PROG


def _make_in_maps(x, W_qkv, W_out):
    in_maps = []
    triu = np.triu(np.ones((128, 128), np.float32)).astype(BF16)
    for c in range(N_CORES):
        b, g = c // 2, c % 2
        heads = [4 * g + i for i in range(HPC)]
        xT = np.ascontiguousarray(x[b].T).astype(BF16)
        # qkT chunk order: [q-pair0, k-pair0, q-pair1, k-pair1]
        rows = []
        for hp in range(2):
            h0, h1 = heads[2 * hp], heads[2 * hp + 1]
            rows.append(W_qkv[64 * h0 : 64 * h0 + 64])  # q of h0
            rows.append(W_qkv[64 * h1 : 64 * h1 + 64])  # q of h1
            rows.append(W_qkv[512 + 64 * h0 : 512 + 64 * h0 + 64])  # k of h0
            rows.append(W_qkv[512 + 64 * h1 : 512 + 64 * h1 + 64])  # k of h1
        W_perm = np.concatenate(rows, axis=0)  # (512, 512)
        wqkT = np.ascontiguousarray(W_perm.T).astype(BF16)
        wv = W_qkv[1024 + 256 * g : 1024 + 256 * (g + 1)]  # (256, 512)
        wvT = np.ascontiguousarray(wv.T).astype(BF16)
        wo = W_out[:, 256 * g : 256 * (g + 1)]  # (512, 256)
        woT = np.ascontiguousarray(wo.T).astype(BF16)
        in_maps.append(
            {"xT": xT, "wqkT": wqkT, "wvT": wvT, "woT": woT, "triu": triu}
        )
    return in_maps


def kernel(x, W_qkv, W_out, b_out):
    from concourse.bass_utils import run_bass_kernel_spmd

    x = np.asarray(x, np.float32)
    W_qkv = np.asarray(W_qkv, np.float32)
    W_out = np.asarray(W_out, np.float32)
    b_out = np.asarray(b_out, np.float32)

    nc = _get_program()
    in_maps = _make_in_maps(x, W_qkv, W_out)
    res = run_bass_kernel_spmd(nc, in_maps, list(range(N_CORES)))
    outs = [r["out"].astype(np.float32) for r in res.results]
    full = np.empty((B, T, D), np.float32)
    for b in range(B):
        full[b] = outs[2 * b] + outs[2 * b + 1] + b_out
    return full



# revision 43
# speedup vs baseline: 1.0295x; 1.0295x over previous
"""Causal self-attention Bass kernel for Trainium2, 8 NeuronCores.

Problem shapes (hardcoded): x (4, 2048, 512), W_qkv (1536, 512),
W_out (512, 512), b_out (512,); NH=8 heads, DH=64.

Sharding: core c handles batch b = c // 2 and head group g = c % 2
(4 heads each). Each core computes its QKV slice, causal attention for
its 4 heads, and a partial output projection over its 256 y-dims.
Host sums the two partials per batch and adds the bias.

Per-core device pipeline (all matmul operands bf16, f32 PSUM accum):
  1. qkT projection: qkT[c_loc, t] chunks ordered [q-pair0, k-pair0,
     q-pair1, k-pair1] so each head's qT/kT share a 64-partition range.
     Only heads 0/1's chunks (m=0,1) run up front; m=2,3 and all v
     quarters are interleaved into head 0's attention loop as PE
     filler for the scores->exp handoff gaps (also keeps the PE dense
     enough that the HAM clock gate stays at full rate longer).
  2. v projection in natural [t, d] layout, augmented with a ones
     column per head (denominator trick); uses ps_s slots so it can
     run while ps_y accumulates.
  3. Per head h, per key chunk j (128 rows): scoresT[k, q] for q >=
     128j only (causality via matmul geometry), one exp ACTIVATE per
     PSUM chunk (<=1024 cols; ACT is the cheapest PSUM->SBUF mover)
     with the 1/8 scale folded in; exp table preloaded via a dummy
     ACTIVATE during the input DMAs. Triangular mask applied to the
     128x128 diagonal block post-exp on the otherwise-idle gpsimd
     engine; av consumes att at a 2-step lag so neither exp nor mask
     latency sits on the PE critical path.
  4. att @ v with lhsT = [v | ones] (M=65): row 64 accumulates the
     softmax denominators for free.
  5. y normalization per 512-aligned column block, emitted as soon as
     the covering avs are in (cols 0:1024 at j==11, overlapping the
     j>=12 attention; the rest at head end). Only the den row is
     copied out; it is cast to bf16 so the rank-1 broadcast matmuls
     run at bf16 rate (fp32 is 4x slower on the PE), and the
     broadcast lands in the already-drained region of ps_y.
  6. Output projection partials (K = 256 local y-dims) stream
     progressively through the last head's tail as their ynormT
     columns become ready; bf16 output halves the store DMA.
"""

import sys

if "/opt/trn_rl_repo" not in sys.path:
    sys.path.insert(0, "/opt/trn_rl_repo")

import numpy as np
import ml_dtypes

B, T, D, NH, DH = 4, 2048, 512, 8, 64
HPC = 4  # heads per core
N_CORES = 8
BF16 = ml_dtypes.bfloat16

_PROG = None


def _build_program():
    import concourse.bass as bass
    import concourse.mybir as mybir
    import concourse.tile as tile
    from concourse import bacc

    f32 = mybir.dt.float32
    bf16 = mybir.dt.bfloat16
    Exp = mybir.ActivationFunctionType.Exp

    nc = bacc.Bacc("TRN2", target_bir_lowering=False, debug=False)

    xT_d = nc.dram_tensor("xT", [D, T], bf16, kind="ExternalInput").ap()
    wqkT_d = nc.dram_tensor("wqkT", [D, 512], bf16, kind="ExternalInput").ap()
    wvT_d = nc.dram_tensor("wvT", [D, 256], bf16, kind="ExternalInput").ap()
    woT_d = nc.dram_tensor("woT", [256, D], bf16, kind="ExternalInput").ap()
    triu_d = nc.dram_tensor("triu", [128, 128], bf16, kind="ExternalInput").ap()
    out_d = nc.dram_tensor("out", [T, D], bf16, kind="ExternalOutput").ap()

    NT = T // 128  # 16 key/query 128-chunks
    NQ = T // 512  # 4 query 512-chunks

    with tile.TileContext(nc) as tc:
        with (
            tc.tile_pool(name="const", bufs=1) as cpool,
            tc.tile_pool(name="att", bufs=2) as apool,
            tc.tile_pool(name="work", bufs=2) as wpool,
            tc.tile_pool(name="outsb", bufs=2) as opool,
            tc.tile_pool(name="ps", bufs=1, space="PSUM") as pspool,
        ):
            # ---- input DMAs ----
            xT_sb = cpool.tile([128, 4, T], bf16, tag="xT")
            for kc in range(4):
                nc.sync.dma_start(out=xT_sb[:, kc, :], in_=xT_d[128 * kc : 128 * (kc + 1), :])
            wqkT_sb = cpool.tile([128, 4, 512], bf16, tag="wqkT")
            for kc in range(4):
                nc.sync.dma_start(out=wqkT_sb[:, kc, :], in_=wqkT_d[128 * kc : 128 * (kc + 1), :])
            wvT_sb = cpool.tile([128, 4, 256], bf16, tag="wvT")
            for kc in range(4):
                nc.sync.dma_start(out=wvT_sb[:, kc, :], in_=wvT_d[128 * kc : 128 * (kc + 1), :])
            woT_sb = cpool.tile([128, 2, 512], bf16, tag="woT")
            for kc in range(2):
                nc.sync.dma_start(out=woT_sb[:, kc, :], in_=woT_d[128 * kc : 128 * (kc + 1), :])
            triu_sb = cpool.tile([128, 128], bf16, tag="triu")
            nc.sync.dma_start(out=triu_sb[:, :], in_=triu_d[:, :])

            # qkT chunks: [q-pair0 | k-pair0 | q-pair1 | k-pair1]; head h at
            # partitions 64*(h%2) .. +64 of chunks (2*(h//2), 2*(h//2)+1).
            qkT_sb = cpool.tile([128, 4, T], bf16, tag="qkT")
            # v_all: per t-chunk, per head: 64 v-dims + a ones column (65).
            v_all = cpool.tile([128, NT, HPC * 65], bf16, tag="v_all")
            ynormT = cpool.tile([128, 2, T], bf16, tag="ynormT")

            ones_ap = v_all[:, :, :].rearrange("p t (h e) -> p (t h) e", e=65)[:, :, 64:65]
            nc.gpsimd.memset(ones_ap, 1.0)
            # ones row at partition 64 used to broadcast the denominator row
            # via a rank-1 fp32 matmul (walrus can't codegen
            # InstPartitionBroadcast; matmul wants lhsT/rhs on the same base
            # partition as the den row, and fp32r inputs would need rounding)
            ones64 = cpool.tile([65, 64], bf16, tag="ones64")
            nc.gpsimd.memset(ones64[:, :], 1.0)

            # ---- phase A: projections. Only m=0,1 (heads 0/1 qk) and the
            # first v half run up front; the rest interleave into head 0's
            # attention as PE filler (they use ps_s slots, never ps_y).
            def emit_proj_m_half(m, half):
                ps = pspool.tile([128, 1024], f32, tag="ps_s", bufs=2)
                for n2 in range(2):
                    n = 2 * half + n2
                    for kc in range(4):
                        nc.tensor.matmul(
                            ps[:, 512 * n2 : 512 * (n2 + 1)],
                            lhsT=wqkT_sb[:, kc, 128 * m : 128 * (m + 1)],
                            rhs=xT_sb[:, kc, 512 * n : 512 * (n + 1)],
                            start=(kc == 0),
                            stop=(kc == 3),
                        )
                nc.vector.tensor_copy(
                    qkT_sb[:, m, 1024 * half : 1024 * (half + 1)], ps[:, :]
                )

            def emit_proj_v_quarter(qt):
                ps = pspool.tile([128, 1024], f32, tag="ps_s", bufs=2)
                for tl in range(4):
                    t = 4 * qt + tl
                    for kc in range(4):
                        nc.tensor.matmul(
                            ps[:, 256 * tl : 256 * (tl + 1)],
                            lhsT=xT_sb[:, kc, 128 * t : 128 * (t + 1)],
                            rhs=wvT_sb[:, kc, :],
                            start=(kc == 0),
                            stop=(kc == 3),
                        )
                dst = v_all[:, 4 * qt : 4 * (qt + 1), :].rearrange(
                    "p t (h e) -> p t h e", e=65
                )[:, :, :, 0:64]
                src = ps[:, :].rearrange("p (t h e) -> p t h e", t=4, h=HPC)
                nc.vector.tensor_copy(dst, src)

            # preload the exp table set during the input DMAs so the first
            # real ACTIVATE doesn't pay the ~2.7us ACT_TABLE_LOAD
            actwarm = wpool.tile([1, 16], f32, tag="actwarm")
            nc.scalar.activation(actwarm[:, :], triu_sb[0:1, 0:16], Exp, scale=0.125)

            emit_proj_m_half(0, 0)
            emit_proj_m_half(0, 1)
            emit_proj_m_half(1, 0)
            emit_proj_m_half(1, 1)

            # ---- phase B/C: per-head attention ----
            def emit_av(h, j, ps_y, att):
                # accumulate yT_aug[:, q] += v_augT @ attT for key chunk j
                q0 = 128 * j
                lhsT = v_all[:, j, :].rearrange("p (h e) -> p h e", e=65)[:, h, :]
                for c in range(NQ):
                    if j > 4 * c + 3:
                        continue  # fully masked: k > all q in this chunk
                    qs = max(512 * c, q0)
                    qe = 512 * (c + 1)
                    nc.tensor.matmul(
                        ps_y[:, qs:qe],
                        lhsT=lhsT,
                        rhs=att[:, qs - q0 : qe - q0],
                        start=(j == 0),
                        stop=(j == 4 * c + 3),
                    )

            def emit_norm_cols(h, c0, c1, ps_y):
                """normalize y[:, c0:c1] (512-aligned) once all avs that
                touch those columns are in (av(j) writes cols >= 128j).
                The den row is cast to bf16 so the rank-1 broadcast
                matmuls run at bf16 rate (fp32 matmuls are 4x slower on
                the PE); the broadcast lands back in the already-copied-
                out region of ps_y."""
                base = 64 * (h % 2)
                w = c1 - c0
                yT = wpool.tile([65, 1024], f32, tag="yT", name=f"yT{h}_{c0}")
                nc.vector.tensor_copy(yT[:, 0:w], ps_y[:, c0:c1])
                den = wpool.tile([1, 1024], bf16, tag="den", name=f"den{h}_{c0}")
                nc.vector.tensor_copy(den[:, 0:w], yT[64:65, 0:w])
                for c2 in range(w // 512):
                    nc.tensor.matmul(
                        ps_y[0:64, c0 + 512 * c2 : c0 + 512 * (c2 + 1)],
                        lhsT=ones64[0:1, :],
                        rhs=den[0:1, 512 * c2 : 512 * (c2 + 1)],
                        start=True,
                        stop=True,
                    )
                recb = wpool.tile([64, 1024], f32, tag="recb", name=f"recb{h}_{c0}")
                nc.vector.reciprocal_approx_fast(
                    out=recb[:, 0:w], in_=ps_y[0:64, c0:c1]
                )
                dst = ynormT[base : base + 64, h // 2, c0:c1]
                nc.vector.tensor_mul(dst, yT[0:64, 0:w], recb[:, 0:w])

            def emit_outproj(grp):
                ps = pspool.tile([128, 1024], f32, tag="ps_s", bufs=2)
                for i2 in range(2):
                    i = 2 * grp + i2
                    for kc in range(2):
                        nc.tensor.matmul(
                            ps[:, 512 * i2 : 512 * (i2 + 1)],
                            lhsT=ynormT[:, kc, 128 * i : 128 * (i + 1)],
                            rhs=woT_sb[:, kc, :],
                            start=(kc == 0),
                            stop=(kc == 1),
                        )
                osb = opool.tile([128, 1024], bf16, tag="osb")
                nc.vector.tensor_copy(osb[:, :], ps[:, :])
                for i2 in range(2):
                    i = 2 * grp + i2
                    nc.sync.dma_start(
                        out=out_d[128 * i : 128 * (i + 1), :],
                        in_=osb[:, 512 * i2 : 512 * (i2 + 1)],
                    )

            for h in range(HPC):
                base = 64 * (h % 2)
                qc = 2 * (h // 2)  # q chunk index; k chunk is qc + 1
                qT = qkT_sb[base : base + 64, qc, :]
                kT = qkT_sb[base : base + 64, qc + 1, :]
                ps_y = pspool.tile([65, T], f32, tag="ps_y")
                att_tiles = {}
                for j in range(NT):
                    q0 = 128 * j
                    att = apool.tile([128, T - q0], bf16, tag=f"att{j}")
                    # split the q range at 1024 so scores(j+1) can run in the
                    # second ps_s buffer while exp(j) drains the first
                    halves = [(q0, 1024), (1024, T)] if j < 8 else [(q0, T)]
                    for hs, he in halves:
                        ps_s = pspool.tile([128, 1024], f32, tag="ps_s", bufs=2)
                        for rel in range(0, he - hs, 512):
                            n = min(512, he - hs - rel)
                            nc.tensor.matmul(
                                ps_s[:, rel : rel + n],
                                lhsT=kT[:, 128 * j : 128 * (j + 1)],
                                rhs=qT[:, hs + rel : hs + rel + n],
                                start=True,
                                stop=True,
                            )
                        nc.scalar.activation(
                            att[:, hs - q0 : he - q0],
                            ps_s[:, 0 : he - hs],
                            Exp,
                            scale=0.125,
                        )
                    # mask the strict upper triangle of the diagonal block
                    nc.gpsimd.tensor_mul(att[:, 0:128], att[:, 0:128], triu_sb[:, :])
                    att_tiles[j] = att
                    if j >= 2:
                        emit_av(h, j - 2, ps_y, att_tiles[j - 2])
                    if h == 0:
                        # deferred projections as PE filler for the
                        # scores->exp ping-pong gaps of the first head
                        if j == 0:
                            emit_proj_v_quarter(0)
                        elif j == 1:
                            emit_proj_v_quarter(1)
                        elif j == 2:
                            emit_proj_m_half(2, 0)
                        elif j == 3:
                            emit_proj_m_half(2, 1)
                        elif j == 4:
                            emit_proj_m_half(3, 0)
                        elif j == 5:
                            emit_proj_m_half(3, 1)
                        elif j == 6:
                            emit_proj_v_quarter(2)
                        elif j == 7:
                            emit_proj_v_quarter(3)
                    if j == 11:
                        # cols 0:1024 of ps_y complete (last write av(7),
                        # emitted at j==9); remaining avs touch cols >=1280
                        emit_norm_cols(h, 0, 1024, ps_y)
                    if h == HPC - 1 and j == 13:
                        emit_outproj(0)
                        emit_outproj(1)
                if h < HPC - 1:
                    emit_av(h, NT - 2, ps_y, att_tiles[NT - 2])
                    emit_av(h, NT - 1, ps_y, att_tiles[NT - 1])
                    emit_norm_cols(h, 1024, 2048, ps_y)
                else:
                    # progressive tail: normalize 512-col quarters as their
                    # avs land so output projections start without waiting
                    # for av(15); grp g consumes ynormT cols 256g..256g+256
                    emit_outproj(2)
                    emit_norm_cols(h, 1024, 1536, ps_y)  # needs av(11)
                    emit_av(h, NT - 2, ps_y, att_tiles[NT - 2])
                    emit_outproj(3)
                    emit_outproj(4)
                    emit_av(h, NT - 1, ps_y, att_tiles[NT - 1])
                    emit_outproj(5)
                    emit_norm_cols(h, 1536, 2048, ps_y)  # needs av(15)
                    emit_outproj(6)
                    emit_outproj(7)

    nc.compile()
    return nc


def _get_program():
    global _PROG
    if _PROG is None:
        _PROG = _build_program()
    return _PROG


def _make_in_maps(x, W_qkv, W_out):
    in_maps = []
    triu = np.triu(np.ones((128, 128), np.float32)).astype(BF16)
    for c in range(N_CORES):
        b, g = c // 2, c % 2
        heads = [4 * g + i for i in range(HPC)]
        xT = np.ascontiguousarray(x[b].T).astype(BF16)
        # qkT chunk order: [q-pair0, k-pair0, q-pair1, k-pair1]
        rows = []
        for hp in range(2):
            h0, h1 = heads[2 * hp], heads[2 * hp + 1]
            rows.append(W_qkv[64 * h0 : 64 * h0 + 64])  # q of h0
            rows.append(W_qkv[64 * h1 : 64 * h1 + 64])  # q of h1
            rows.append(W_qkv[512 + 64 * h0 : 512 + 64 * h0 + 64])  # k of h0
            rows.append(W_qkv[512 + 64 * h1 : 512 + 64 * h1 + 64])  # k of h1
        W_perm = np.concatenate(rows, axis=0)  # (512, 512)
        wqkT = np.ascontiguousarray(W_perm.T).astype(BF16)
        wv = W_qkv[1024 + 256 * g : 1024 + 256 * (g + 1)]  # (256, 512)
        wvT = np.ascontiguousarray(wv.T).astype(BF16)
        wo = W_out[:, 256 * g : 256 * (g + 1)]  # (512, 256)
        woT = np.ascontiguousarray(wo.T).astype(BF16)
        in_maps.append(
            {"xT": xT, "wqkT": wqkT, "wvT": wvT, "woT": woT, "triu": triu}
        )
    return in_maps


def kernel(x, W_qkv, W_out, b_out):
    from concourse.bass_utils import run_bass_kernel_spmd

    x = np.asarray(x, np.float32)
    W_qkv = np.asarray(W_qkv, np.float32)
    W_out = np.asarray(W_out, np.float32)
    b_out = np.asarray(b_out, np.float32)

    nc = _get_program()
    in_maps = _make_in_maps(x, W_qkv, W_out)
    res = run_bass_kernel_spmd(nc, in_maps, list(range(N_CORES)))
    outs = [r["out"].astype(np.float32) for r in res.results]
    full = np.empty((B, T, D), np.float32)
    for b in range(B):
        full[b] = outs[2 * b] + outs[2 * b + 1] + b_out
    return full

